# revision 1
# baseline (speedup 1.0000x reference)
"""Causal self-attention (B=128, T=512, C=512, H=16) on 8 Trainium2 NeuronCores.

Sharding: data-parallel over batch — each core computes 16 of the 128
batch elements end-to-end; weights are replicated. No collectives.

Per-core kernel (Bass/Tile; matmul operands in bf16, fp32 accumulation):
  - x is transposed on-chip via the PE (identity matmul) so C sits on
    partitions.
  - qT/kT [f, t] = W_attn.T @ x.T with W_attn natural as the stationary
    operand; head h lands on partition rows 32*(h%4) of f-block h//4,
    giving 4-way PE row-group packing for the K=32 scores matmuls.
  - scores are computed transposed, S'[k, q] = kT.T @ qT, over causal
    column ranges padded to >=256 (fp32r needs N>=256 for 1 cyc/row and
    even N always).
  - E' = exp(S'/sqrt(D)) on ScalarE (scale fused); causal masking via
    gpsimd affine_select directly on the fp32r tiles.
  - yT[d, q] = [v | 1].T @ E' accumulated over k-blocks in PSUM; the ones
    column produces the softmax denominator in row 32 of the same PSUM.
  - normalization: DVE reciprocal of the denominator row, K=1 ones-matmul
    broadcast across partitions, DVE multiply; SBUF->SBUF DMA moves each
    head's 32 rows into the packed yT layout.
  - out = yT.T @ W_proj + b_proj streamed straight back to DRAM.

All fp32r-matmul operands are produced by compute ops writing
float32r-typed tiles (the walrus BIR verifier rejects unrounded fp32
producers feeding fp32r matmuls).
"""

import math
import sys

if "/opt/trn_rl_repo" not in sys.path:
    sys.path.insert(0, "/opt/trn_rl_repo")

import numpy as np

import concourse.tile as tile
from concourse import bacc, mybir
from concourse.bass_utils import run_bass_kernel_spmd
from concourse.masks import make_identity

F32 = mybir.dt.float32
F32R = mybir.dt.float32r
BF16 = mybir.dt.float16  # fp16: full matmul rate + FWL, 11-bit mantissa
AF = mybir.ActivationFunctionType
ALU = mybir.AluOpType

B, T, C, H, D = 128, 512, 512, 16, 32
P = 128
N_CORES = 8
BC = B // N_CORES       # 16 batches per core
NCBLK = C // P          # 4
NTBLK = T // P          # 4
HPG = 4                 # heads per 128-row f-block
NHG = H // HPG          # 4 head groups
SCALE = 1.0 / math.sqrt(D)
QR0 = [0, 128, 256, 384]  # q range start per k-block (exact causal)


def _build_kernel(tc, out, x, w_attn, b_attn, w_proj, b_proj, b_count=BC):
    nc = tc.nc
    with (
        tc.tile_pool(name="const", bufs=1) as const_pool,
        tc.tile_pool(name="xnat", bufs=4) as xnat_pool,
        tc.tile_pool(name="xt", bufs=2) as xt_pool,
        tc.tile_pool(name="qt", bufs=2) as qt_pool,
        tc.tile_pool(name="kt", bufs=2) as kt_pool,
        tc.tile_pool(name="vaug", bufs=2) as v_pool,
        tc.tile_pool(name="eprime", bufs=34) as e_pool,
        tc.tile_pool(name="ytn", bufs=2) as ytn_pool,
        tc.tile_pool(name="small", bufs=2) as small_pool,
        tc.tile_pool(name="obuf", bufs=3) as o_pool,
        tc.tile_pool(name="s_psum", bufs=4, space="PSUM") as s_psum,
        tc.tile_pool(name="y_psum", bufs=2, space="PSUM") as y_psum,
        tc.tile_pool(name="mm_psum", bufs=2, space="PSUM") as mm_psum,
    ):
        identity = const_pool.tile([P, P], F32, tag="identity")
        make_identity(nc, identity)
        onesf = const_pool.tile([P, P], F32, tag="onesf")
        nc.gpsimd.memset(onesf[:], 1.0)
        ones_r = const_pool.tile([P, P], BF16, tag="ones_r")
        nc.vector.tensor_copy(ones_r[:], onesf[:])

        # weights: DMA to fp32 staging, then round into fp32r tiles.
        # The staging pool is scoped so its SBUF is released before the
        # batch loop pools are sized.
        wqk = const_pool.tile([P, NCBLK, 2 * C], BF16, tag="wqk")
        wv = const_pool.tile([P, NCBLK, C], BF16, tag="wv")
        wp = const_pool.tile([P, NCBLK, C], BF16, tag="wp")
        battn = const_pool.tile([P, 8], F32, tag="battn")
        bv_row = const_pool.tile([1, C], BF16, tag="bv_row")
        bp_row = const_pool.tile([1, C], BF16, tag="bp_row")
        with tc.tile_pool(name="stage", bufs=2) as stage_pool:
            for co in range(NCBLK):
                st = stage_pool.tile([P, 2 * C], F32, tag="stage", name="st_qk")
                nc.sync.dma_start(
                    st[:],
                    w_attn[co * P : (co + 1) * P, : 2 * C],
                )
                nc.vector.tensor_copy(wqk[:, co, :], st[:])
                st2 = stage_pool.tile([P, 2 * C], F32, tag="stage", name="st_v")
                nc.sync.dma_start(
                    st2[:, :C], w_attn[co * P : (co + 1) * P, 2 * C :]
                )
                nc.vector.tensor_copy(wv[:, co, :], st2[:, :C])
                st3 = stage_pool.tile([P, 2 * C], F32, tag="stage", name="st_p")
                nc.sync.dma_start(st3[:, :C], w_proj[co * P : (co + 1) * P, :])
                nc.vector.tensor_copy(wp[:, co, :], st3[:, :C])

            nc.sync.dma_start(
                battn[:], b_attn[: 2 * C].rearrange("(g p) -> p g", p=P)
            )
            st4 = stage_pool.tile([P, 2 * C], F32, tag="stage", name="st_b")
            nc.sync.dma_start(
                st4[0:1, :C], b_attn[2 * C :].rearrange("(o f) -> o f", o=1)
            )
            nc.sync.dma_start(
                st4[0:1, C : 2 * C], b_proj.rearrange("(o f) -> o f", o=1)
            )
            nc.vector.tensor_copy(bv_row[:], st4[0:1, :C])
            nc.vector.tensor_copy(bp_row[:], st4[0:1, C : 2 * C])

        bv_bcast = const_pool.tile([P, C], F32, tag="bv_bcast")
        bp_bcast = const_pool.tile([P, C], F32, tag="bp_bcast")
        for row, bcast in ((bv_row, bv_bcast), (bp_row, bp_bcast)):
            ps = mm_psum.tile([P, C], F32, tag="mm", name="bps")
            nc.tensor.matmul(ps[:], ones_r[0:1, :], row[:], start=True, stop=True)
            nc.scalar.copy(bcast[:], ps[:])

        def emit_proj(pb, ytn_t):
            # output projection for batch pb (deferred one iteration so the
            # in-order PE queue never stalls on the normalize/DMA chain)
            for tb in range(NTBLK):
                ps = mm_psum.tile([P, C], F32, tag="mm", name="ops")
                for cb in range(NCBLK):
                    nc.tensor.matmul(
                        ps[:],
                        ytn_t[:, cb, tb * P : (tb + 1) * P],
                        wp[:, cb, :],
                        start=(cb == 0),
                        stop=(cb == NCBLK - 1),
                    )
                ob = o_pool.tile([P, C], F32, tag="obuf")
                nc.vector.tensor_tensor(ob[:], ps[:], bp_bcast[:], ALU.add)
                nc.sync.dma_start(out[pb, tb * P : (tb + 1) * P, :], ob[:])

        prev_proj = None
        for b in range(b_count):
            # transpose x_b -> xT_b [c, t] (fp32 PE transpose, cast on evac)
            xt_b = xt_pool.tile([P, NCBLK, T], BF16, tag="xt")
            for tb in range(NTBLK):
                xn = xnat_pool.tile([P, C], F32, tag="xnat")
                nc.sync.dma_start(xn[:], x[b, tb * P : (tb + 1) * P, :])
                for cb in range(NCBLK):
                    tps = mm_psum.tile([P, P], F32, tag="mm", name="tps")
                    nc.tensor.transpose(
                        tps[:], xn[:, cb * P : (cb + 1) * P], identity[:]
                    )
                    nc.vector.tensor_copy(xt_b[:, cb, tb * P : (tb + 1) * P], tps[:])

            # QKV
            qt_b = qt_pool.tile([P, NHG, T], BF16, tag="qt")
            kt_b = kt_pool.tile([P, NHG, T], BF16, tag="kt")
            v_b = v_pool.tile([P, NTBLK, H, 2 * D], BF16, tag="vaug")
            nc.vector.tensor_copy(
                v_b[:, :, :, 0:D],
                onesf[:, 0:D]
                .rearrange("p (a h d) -> p a h d", a=1, h=1, d=D)
                .to_broadcast([P, NTBLK, H, D]),
            )

            for g in range(2 * NHG):
                ps = mm_psum.tile([P, T], F32, tag="mm", name="qkps")
                for co in range(NCBLK):
                    nc.tensor.matmul(
                        ps[:],
                        wqk[:, co, g * P : (g + 1) * P],
                        xt_b[:, co, :],
                        start=(co == 0),
                        stop=(co == NCBLK - 1),
                    )
                dst = qt_b[:, g, :] if g < NHG else kt_b[:, g - NHG, :]
                nc.vector.tensor_scalar_add(dst, ps[:], battn[:, g : g + 1])

            for tb in range(NTBLK):
                ps = mm_psum.tile([P, C], F32, tag="mm", name="vps")
                for co in range(NCBLK):
                    nc.tensor.matmul(
                        ps[:],
                        xt_b[:, co, tb * P : (tb + 1) * P],
                        wv[:, co, :],
                        start=(co == 0),
                        stop=(co == NCBLK - 1),
                    )
                nc.vector.tensor_tensor(
                    v_b[:, tb, :, D : 2 * D],
                    ps.rearrange("p (h d) -> p h d", h=H),
                    bv_bcast.rearrange("p (h d) -> p h d", h=H),
                    ALU.add,
                )

            if prev_proj is not None:
                emit_proj(*prev_proj)

            # attention — software-pipelined: scores/exp for head pair N
            # overlap yT/normalize for pair N-1, keeping the PE stream dense
            # (HAM stays un-throttled).
            ytn_b = ytn_pool.tile([P, NCBLK, T], BF16, tag="ytn")

            def emit_yt(g, pair, e_tiles):
                # 2-way column-packed: heads (hp, hp+1) share one PSUM bank
                # via tile_position (0,0)/(0,64); denom rows 0 and 64.
                for hp0 in range(0, len(pair), 2):
                    yps = y_psum.tile([P, T], F32, tag="y", name="yps")
                    for i in range(NTBLK):
                        for lo in (0, 1):
                            hp = pair[hp0 + lo]
                            h = g * HPG + hp
                            nc.tensor.matmul(
                                yps[64 * lo : 64 * lo + 2 * D, QR0[i] : T],
                                v_b[:, i, h, :],
                                e_tiles[hp, i],
                                start=(i == 0),
                                stop=(i == NTBLK - 1),
                                tile_position=(0, 64 * lo),
                                skip_group_check=True,
                            )
                    for lo in (0, 1):
                        hp = pair[hp0 + lo]
                        base = 64 * lo
                        den = small_pool.tile([1, T], F32, tag="den")
                        if lo:
                            # custom DVE ops read base partition 0; stage row 64
                            nc.vector.tensor_copy(den[0:1, :], yps[base : base + 1, :])
                        else:
                            den = yps
                        rec = small_pool.tile([1, T], F32, tag="rec")
                        nc.vector.reciprocal_approx_fast(rec[0:1, :], den[0:1, :])
                        bcast_sb = small_pool.tile([D, T], F32, tag="bcast")
                        nc.gpsimd.partition_broadcast(
                            bcast_sb[:], rec[0:1, :], channels=D
                        )
                        ytmp = small_pool.tile([D, T], BF16, tag="ytmp")
                        nc.vector.tensor_tensor(
                            ytmp[:], yps[base + D : base + 2 * D, :], bcast_sb[:], ALU.mult
                        )
                        nc.sync.dma_start(
                            ytn_b[32 * hp : 32 * hp + 32, g, :], ytmp[:]
                        )

            pending = None
            for g in range(NHG):
                for pair in ((0, 1, 2, 3),):
                    e_tiles = {}
                    for i in range(NTBLK):
                        q0 = QR0[i]
                        n = T - q0
                        for hp in pair:
                            sps_t = s_psum.tile([P, T], F32, tag="s", name="sps")
                            sps = sps_t[:, :n]
                            nc.tensor.matmul(
                                sps,
                                kt_b[32 * hp : 32 * hp + 32, g, i * P : (i + 1) * P],
                                qt_b[32 * hp : 32 * hp + 32, g, q0:T],
                                start=True,
                                stop=True,
                                tile_position=(32 * hp, 0),
                            )
                            ep_t = e_pool.tile([P, T], BF16, tag="e", name="ep")
                            ep = ep_t[:, :n]
                            nc.scalar.activation(ep, sps, AF.Exp, scale=SCALE)
                            # causal mask on the diagonal block (local cols
                            # [0, 128)): keep iff f - p >= 0.
                            nc.gpsimd.affine_select(
                                out=ep[:, 0:P],
                                in_=ep[:, 0:P],
                                compare_op=ALU.is_ge,
                                fill=0.0,
                                base=0,
                                channel_multiplier=-1,
                                pattern=[[1, P]],
                            )
                            e_tiles[hp, i] = ep
                    if pending is not None:
                        emit_yt(*pending)
                    pending = (g, pair, e_tiles)
            emit_yt(*pending)
            prev_proj = (b, ytn_b)

        emit_proj(*prev_proj)



_NC_CACHE = None


def build_nc(b_count: int = BC, num_devices: int = N_CORES):
    nc = bacc.Bacc(
        "TRN2", target_bir_lowering=False, debug=False, num_devices=num_devices
    )
    x = nc.dram_tensor("x", [b_count, T, C], F32, kind="ExternalInput").ap()
    w_attn = nc.dram_tensor("w_attn", [C, 3 * C], F32, kind="ExternalInput").ap()
    b_attn = nc.dram_tensor("b_attn", [3 * C], F32, kind="ExternalInput").ap()
    w_proj = nc.dram_tensor("w_proj", [C, C], F32, kind="ExternalInput").ap()
    b_proj = nc.dram_tensor("b_proj", [C], F32, kind="ExternalInput").ap()
    out = nc.dram_tensor("out", [b_count, T, C], F32, kind="ExternalOutput").ap()
    with tile.TileContext(nc) as tc:
        _build_kernel(tc, out, x, w_attn, b_attn, w_proj, b_proj, b_count)
    nc.compile()
    return nc


def _get_nc():
    global _NC_CACHE
    if _NC_CACHE is None:
        _NC_CACHE = build_nc(BC, N_CORES)
    return _NC_CACHE


def kernel(x, W_attn, b_attn, W_proj, b_proj):
    x = np.ascontiguousarray(np.asarray(x, dtype=np.float32))
    W_attn = np.ascontiguousarray(np.asarray(W_attn, dtype=np.float32))
    b_attn = np.ascontiguousarray(np.asarray(b_attn, dtype=np.float32))
    W_proj = np.ascontiguousarray(np.asarray(W_proj, dtype=np.float32))
    b_proj = np.ascontiguousarray(np.asarray(b_proj, dtype=np.float32))

    nc = _get_nc()
    in_maps = [
        {
            "x": x[c * BC : (c + 1) * BC],
            "w_attn": W_attn,
            "b_attn": b_attn,
            "w_proj": W_proj,
            "b_proj": b_proj,
        }
        for c in range(N_CORES)
    ]
    res = run_bass_kernel_spmd(nc, in_maps, core_ids=list(range(N_CORES)))
    return np.concatenate([res.results[c]["out"] for c in range(N_CORES)], axis=0)

